# revision 14
# baseline (speedup 1.0000x reference)
"""TRN2 Bass kernel for ConvNeXt-MLP + parallel top-2-of-3 LoRA-MoE.

Data-parallel over the token dim across 8 NeuronCores (12544 tokens ->
1568/core). All weights replicated. Per core, everything is computed in
feature-major ("transposed") layout: activations live in SBUF as
[features_on_partitions, tokens_on_free_dim]; the host transposes x in and
the output back out.

Math per core (T = 1568 tokens):
  base:   outT = w2^T @ gelu(w1^T @ xT + b1) + b2          (f32r matmuls)
  router: logitsT = rw^T @ xT + rb (exact f32 on PE), transposed to
          token-major in 128-token chunks, softmax + top-2-of-3 +
          renormalize as dense per-expert weights, transposed back
  lora:   actT = gelu(wd^T @ xT); scaled = actT * expand(comb);
          moeT = wu^T @ scaled, accumulated into the same PSUM as the base

Hidden dim (3072 = 24 chunks) is processed in 4 groups of 6 chunks so that
w1/w2 stream through SBUF exactly once; the output accumulates per group in
6 PSUM banks and across groups in SBUF.
"""

import numpy as np

import concourse.bacc as bacc
import concourse.mybir as mybir
import concourse.tile as tile
from concourse.bass_utils import run_bass_kernel_spmd

F32 = mybir.dt.float32
F32R = mybir.dt.float32r
AF = mybir.ActivationFunctionType
ALU = mybir.AluOpType
AX = mybir.AxisListType

NCORES = 8
B, N, D = 64, 196, 768
T = B * N                  # 12544 tokens total
TC = T // NCORES           # 1568 tokens per core
HID = 4 * D                # 3072
E, R = 3, 8
ER = E * R                 # 24
DC = D // 128              # 6 input-feature chunks
HC = HID // 128            # 24 hidden chunks
MC = D // 128              # 6 output chunks
NGROUPS = 4
GH = HC // NGROUPS         # 6 hidden chunks per group
NT_SIZES = [392, 392, 392, 392]    # token tiles per core (sum = 1568)
RC_SIZES = [128] * 12 + [32]       # router token chunks (sum = 1568)

_cache = {}


def _build():
    nc = bacc.Bacc("TRN2", target_bir_lowering=False, debug=False)

    xt_d = nc.dram_tensor("xt", [D, TC], F32R, kind="ExternalInput")
    w1_d = nc.dram_tensor("w1", [D, HID], F32R, kind="ExternalInput")
    w2_d = nc.dram_tensor("w2", [HID, D], F32R, kind="ExternalInput")
    wu_d = nc.dram_tensor("wu", [ER, D], F32R, kind="ExternalInput")
    b1_d = nc.dram_tensor("b1r", [128, HC], F32, kind="ExternalInput")
    b2_d = nc.dram_tensor("b2r", [128, MC], F32, kind="ExternalInput")
    rwd_d = nc.dram_tensor("rwd", [D, 56], F32, kind="ExternalInput")
    rb_d = nc.dram_tensor("rb", [E, 1], F32, kind="ExternalInput")
    bx_d = nc.dram_tensor("bexp", [E, ER], F32R, kind="ExternalInput")
    id_d = nc.dram_tensor("ident", [128, 128], F32, kind="ExternalInput")
    out_d = nc.dram_tensor("outT", [D, TC], F32, kind="ExternalOutput")

    with tile.TileContext(nc) as tc:
        with (
            tc.tile_pool(name="const", bufs=1) as cp,
            tc.tile_pool(name="big", bufs=1) as bp,
            tc.tile_pool(name="wts", bufs=2) as wp,
            tc.tile_pool(name="hbuf", bufs=3) as hp,
            tc.tile_pool(name="small", bufs=2) as sp,
        ):
            # ---- resident loads ----
            # small constants go on the gpsimd DMA queue so they don't sit
            # behind the 4.8MB xt stream on the sync queue
            rwd = cp.tile([128, DC * 56], F32, tag="rwd")
            nc.gpsimd.dma_start(
                rwd[:].rearrange("p (c e) -> p c e", c=DC),
                rwd_d.rearrange("(c p) e -> p c e", p=128),
            )
            wu = cp.tile([ER, D], F32R, tag="wu")
            nc.gpsimd.dma_start(wu[:], wu_d[:])
            b1 = cp.tile([128, HC], F32, tag="b1")
            nc.gpsimd.dma_start(b1[:], b1_d[:])
            b2 = cp.tile([128, MC], F32, tag="b2")
            nc.gpsimd.dma_start(b2[:], b2_d[:])
            rb = cp.tile([E, 1], F32, tag="rb")
            nc.gpsimd.dma_start(rb[:], rb_d[:])
            bx = cp.tile([E, ER], F32R, tag="bx")
            nc.gpsimd.dma_start(bx[:], bx_d[:])
            ident = cp.tile([128, 128], F32, tag="ident")
            nc.gpsimd.dma_start(ident[:], id_d[:])

            # xt as one tile per token tile so deps are per token range
            xts = []
            t0 = 0
            for i, n in enumerate(NT_SIZES):
                x_i = bp.tile([128, DC * n], F32R, tag=f"xt{i}",
                              name=f"xt{i}")
                h = n // 2
                for lo, hi in ((0, h), (h, n)):
                    nc.sync.dma_start(
                        x_i[:].rearrange("p (c t) -> p c t", c=DC)[:, :, lo:hi],
                        xt_d.rearrange("(c p) t -> p c t", p=128)
                            [:, :, t0 + lo:t0 + hi],
                    )
                xts.append(x_i)
                t0 += n

            lgT = bp.tile([E, TC], F32, tag="lgT")
            comb_t = bp.tile([E, TC], F32R, tag="combt")
            scaled = bp.tile([ER, TC], F32R, tag="scaled")
            acc = bp.tile([128, MC * TC], F32, tag="acc")

            # ---- phase A: router (exact f32) + LoRA expert activations ----
            # PE order: logits MMs -> logit transposes -> LoRA down MMs ->
            # comb transposes -> expand MMs. The DVE softmax chains overlap
            # the LoRA matmuls so the PE never waits on them.
            lgtok = bp.tile([128, 3 * len(RC_SIZES)], F32, tag="lgtok")
            combtok = bp.tile([128, 3 * len(RC_SIZES)], F32, tag="combtok")
            acts = bp.tile([ER, TC], F32, tag="acts")
            with tc.tile_pool(name="psA", bufs=1, space="PSUM") as psA:
                # merged router logits + LoRA down, exact f32:
                # [3+24, n] = [rw | wd]^T @ xT
                t0 = 0
                for i, n in enumerate(NT_SIZES):
                    dn27 = psA.tile([56, 512], F32, tag="dn27", bufs=2,
                                    name=f"dn27_{t0}")
                    for c in range(DC):
                        nc.tensor.matmul(
                            dn27[:, :n],
                            rwd[:, c * 56:(c + 1) * 56],
                            xts[i][:, c * n:(c + 1) * n].bitcast(F32),
                            start=(c == 0), stop=(c == DC - 1),
                        )
                    nc.vector.tensor_scalar_add(lgT[:, t0:t0 + n],
                                                dn27[:E, :n], rb[:])
                    nc.scalar.activation(acts[:, t0:t0 + n], dn27[32:, :n],
                                         AF.Gelu)
                    t0 += n

                # transpose logits to token-major, 128-token chunks
                t0 = 0
                for ci, n in enumerate(RC_SIZES):
                    lg = psA.tile([128, E], F32, tag="lg", bufs=2,
                                  name=f"lg_{t0}")
                    nc.tensor.transpose(lg[:n, :], lgT[:, t0:t0 + n],
                                        ident[:E, :E])
                    nc.vector.tensor_copy(lgtok[:n, 3 * ci:3 * ci + 3],
                                          lg[:n, :])
                    t0 += n

                # softmax + top-2-of-3 combine weights (DVE/ACT only)
                t0 = 0
                for ci, n in enumerate(RC_SIZES):
                    lg = lgtok[:, 3 * ci:3 * ci + 3]
                    mx = sp.tile([128, 1], F32, tag="mx", name=f"mx_{t0}")
                    nc.vector.tensor_reduce(mx[:n], lg[:n, :], axis=AX.X,
                                            op=ALU.max)
                    nmx = sp.tile([128, 1], F32, tag="nmx", name=f"nmx_{t0}")
                    nc.vector.tensor_scalar_mul(nmx[:n], mx[:n], -1.0)
                    probs = sp.tile([128, E], F32, tag="probs",
                                    name=f"probs_{t0}")
                    ssum = sp.tile([128, 1], F32, tag="ssum", name=f"ssum_{t0}")
                    nc.scalar.activation(probs[:n, :], lg[:n, :], AF.Exp,
                                         bias=nmx[:n], accum_out=ssum[:n])
                    pmin = sp.tile([128, 1], F32, tag="pmin", name=f"pmin_{t0}")
                    nc.vector.tensor_reduce(pmin[:n], probs[:n, :], axis=AX.X,
                                            op=ALU.min)
                    rs = sp.tile([128, 1], F32, tag="rs", name=f"rs_{t0}")
                    nc.vector.reciprocal(rs[:n], ssum[:n])
                    d0 = sp.tile([128, 1], F32, tag="d0", name=f"d0_{t0}")
                    nc.vector.tensor_sub(d0[:n], ssum[:n], pmin[:n])
                    den = sp.tile([128, 1], F32, tag="den", name=f"den_{t0}")
                    nc.vector.tensor_scalar(den[:n], d0[:n], rs[:n], 1e-6,
                                            op0=ALU.mult, op1=ALU.add)
                    invd = sp.tile([128, 1], F32, tag="invd", name=f"invd_{t0}")
                    nc.vector.reciprocal(invd[:n], den[:n])
                    t1 = sp.tile([128, 1], F32, tag="t1", name=f"t1_{t0}")
                    nc.vector.tensor_mul(t1[:n], rs[:n], invd[:n])
                    mask = sp.tile([128, E], F32, tag="mask", name=f"mask_{t0}")
                    nc.vector.tensor_scalar(mask[:n, :], probs[:n, :], pmin[:n],
                                            None, op0=ALU.is_gt)
                    nc.vector.scalar_tensor_tensor(
                        combtok[:n, 3 * ci:3 * ci + 3], mask[:n, :], t1[:n],
                        probs[:n, :], op0=ALU.mult, op1=ALU.mult,
                    )
                    t0 += n

            # ---- phase B: base MLP + LoRA-up, hidden dim in 4 groups ----
            with (
                tc.tile_pool(name="psO", bufs=1, space="PSUM") as psO,
                tc.tile_pool(name="psH", bufs=2, space="PSUM") as psH,
            ):
                for g in range(NGROUPS):
                    w1g = wp.tile([128, DC * GH * 128], F32R, tag="w1g")
                    nc.sync.dma_start(
                        w1g[:].rearrange("p (c f) -> p c f", c=DC),
                        w1_d.rearrange("(c p) f -> p c f", p=128)
                            [:, :, g * GH * 128:(g + 1) * GH * 128],
                    )
                    w2g = wp.tile([128, GH * D], F32R, tag="w2g")
                    nc.sync.dma_start(
                        w2g[:].rearrange("p (c f) -> p c f", c=GH),
                        w2_d.rearrange("(c p) f -> p c f", p=128)
                            [:, g * GH:(g + 1) * GH, :],
                    )
                    t0 = 0
                    for nt, n in enumerate(NT_SIZES):
                        outp = [psO.tile([128, 512], F32, tag=f"out{m}",
                                         name=f"out{m}_{g}_{nt}")
                                for m in range(MC)]
                        hsb = [None] * GH
                        for j in range(GH + 1):
                            if j < GH:
                                hps = psH.tile([128, 512], F32, tag="h",
                                               name=f"h_{g}_{nt}_{j}")
                                for c in range(DC):
                                    nc.tensor.matmul(
                                        hps[:, :n],
                                        w1g[:, (c * GH + j) * 128:
                                               (c * GH + j) * 128 + 128],
                                        xts[nt][:, c * n:(c + 1) * n],
                                        start=(c == 0), stop=(c == DC - 1),
                                    )
                                hsb[j] = hp.tile([128, 512], F32R, tag="hs",
                                                 name=f"hs_{g}_{nt}_{j}")
                                nc.scalar.activation(
                                    hsb[j][:, :n], hps[:, :n], AF.Gelu,
                                    bias=b1[:, g * GH + j:g * GH + j + 1],
                                )
                            if j >= 1:
                                jj = j - 1
                                for m in range(MC):
                                    nc.tensor.matmul(
                                        outp[m][:, :n],
                                        w2g[:, jj * D + m * 128:
                                               jj * D + m * 128 + 128],
                                        hsb[jj][:, :n],
                                        start=(jj == 0),
                                        stop=(jj == GH - 1 and g < NGROUPS - 1),
                                    )
                        if g == NGROUPS - 1:
                            for m in range(MC):
                                nc.tensor.matmul(
                                    outp[m][:, :n],
                                    wu[:, m * 128:(m + 1) * 128],
                                    scaled[:, t0:t0 + n],
                                    start=False, stop=True,
                                )
                        for m in range(MC):
                            a = acc[:, m * TC + t0:m * TC + t0 + n]
                            if g == 0:
                                nc.vector.tensor_copy(a, outp[m][:, :n])
                            elif g < NGROUPS - 1:
                                nc.vector.tensor_add(a, a, outp[m][:, :n])
                            else:
                                nc.vector.scalar_tensor_tensor(
                                    a, outp[m][:, :n], b2[:, m:m + 1], a,
                                    op0=ALU.add, op1=ALU.add,
                                )
                                nc.sync.dma_start(
                                    out_d[m * 128:(m + 1) * 128, t0:t0 + n], a)
                        t0 += n

                    if g == 0:
                        # comb transposes + expand, overlapped behind group 0
                        tq = 0
                        for ci, n in enumerate(RC_SIZES):
                            tp = psH.tile([E, 128], F32, tag="h",
                                          name=f"tp_{tq}")
                            nc.tensor.transpose(tp[:, :n],
                                                combtok[:n, 3 * ci:3 * ci + 3],
                                                ident[:n, :n])
                            nc.vector.tensor_copy(comb_t[:, tq:tq + n],
                                                  tp[:, :n])
                            tq += n
                        tq = 0
                        for i, n in enumerate(NT_SIZES):
                            ex = psH.tile([ER, 512], F32, tag="h",
                                          name=f"ex_{tq}")
                            nc.tensor.matmul(ex[:, :n], bx[:],
                                             comb_t[:, tq:tq + n],
                                             start=True, stop=True)
                            nc.vector.tensor_mul(scaled[:, tq:tq + n],
                                                 acts[:, tq:tq + n], ex[:, :n])
                            tq += n

    nc.compile()
    return nc


def _pack_rwd(router_w, w_down):
    rwd = np.zeros((D, 56), np.float32)
    rwd[:, :E] = np.asarray(router_w, np.float32)
    rwd[:, 32:] = np.asarray(w_down, np.float32).transpose(1, 0, 2).reshape(D, ER)
    return rwd


def _prep_inputs(x, w1, b1, w2, b2, router_w, router_b, w_down, w_up):
    x = np.ascontiguousarray(np.asarray(x, dtype=np.float32))
    xT = x.reshape(T, D).T  # [D, T]
    common = {
        "w1": np.ascontiguousarray(np.asarray(w1, np.float32)),
        "w2": np.ascontiguousarray(np.asarray(w2, np.float32)),
        "wu": np.ascontiguousarray(np.asarray(w_up, np.float32).reshape(ER, D)),
        "b1r": np.ascontiguousarray(
            np.asarray(b1, np.float32).reshape(HC, 128).T),
        "b2r": np.ascontiguousarray(
            np.asarray(b2, np.float32).reshape(MC, 128).T),
        "rwd": _pack_rwd(router_w, w_down),
        "rb": np.ascontiguousarray(
            np.asarray(router_b, np.float32).reshape(E, 1)),
        "bexp": np.repeat(np.eye(E, dtype=np.float32), R, axis=1),
        "ident": np.eye(128, dtype=np.float32),
    }
    in_maps = []
    for c in range(NCORES):
        m = dict(common)
        m["xt"] = np.ascontiguousarray(xT[:, c * TC:(c + 1) * TC])
        in_maps.append(m)
    return in_maps


def _run(inputs, trace=False):
    if "nc" not in _cache:
        _cache["nc"] = _build()
    nc = _cache["nc"]
    in_maps = _prep_inputs(**inputs)
    res = run_bass_kernel_spmd(nc, in_maps, core_ids=list(range(NCORES)),
                               trace=trace)
    outT = np.concatenate([res.results[c]["outT"] for c in range(NCORES)],
                          axis=1)  # [D, T]
    out = np.ascontiguousarray(outT.T).reshape(B, N, D).astype(np.float32)
    return out, res


def kernel(**inputs):
    return _run(inputs)[0]


# revision 15
# speedup vs baseline: 1.0421x; 1.0421x over previous
"""TRN2 Bass kernel for ConvNeXt-MLP + parallel top-2-of-3 LoRA-MoE.

Data-parallel over the token dim across 8 NeuronCores (12544 tokens ->
1568/core). All weights replicated. Per core, everything is computed in
feature-major ("transposed") layout: activations live in SBUF as
[features_on_partitions, tokens_on_free_dim]; the host transposes x in and
the output back out.

Math per core (T = 1568 tokens):
  base:   outT = w2^T @ gelu(w1^T @ xT + b1) + b2          (f32r matmuls)
  router: logitsT = rw^T @ xT + rb (exact f32 on PE), transposed to
          token-major in 128-token chunks, softmax + top-2-of-3 +
          renormalize as dense per-expert weights, transposed back
  lora:   actT = gelu(wd^T @ xT); scaled = actT * expand(comb);
          moeT = wu^T @ scaled, accumulated into the same PSUM as the base

Hidden dim (3072 = 24 chunks) is processed in 4 groups of 6 chunks so that
w1/w2 stream through SBUF exactly once; the output accumulates per group in
6 PSUM banks and across groups in SBUF.
"""

import numpy as np

import concourse.bacc as bacc
import concourse.mybir as mybir
import concourse.tile as tile
from concourse.bass_utils import run_bass_kernel_spmd

F32 = mybir.dt.float32
F32R = mybir.dt.float32r
AF = mybir.ActivationFunctionType
ALU = mybir.AluOpType
AX = mybir.AxisListType

NCORES = 8
B, N, D = 64, 196, 768
T = B * N                  # 12544 tokens total
TC = T // NCORES           # 1568 tokens per core
HID = 4 * D                # 3072
E, R = 3, 8
ER = E * R                 # 24
DC = D // 128              # 6 input-feature chunks
HC = HID // 128            # 24 hidden chunks
MC = D // 128              # 6 output chunks
NGROUPS = 4
GH = HC // NGROUPS         # 6 hidden chunks per group
NT_SIZES = [392, 392, 392, 392]    # token tiles per core (sum = 1568)
RC_SIZES = [128] * 12 + [32]       # router token chunks (sum = 1568)

_cache = {}


def _build():
    nc = bacc.Bacc("TRN2", target_bir_lowering=False, debug=False)

    xt_d = nc.dram_tensor("xt", [D, TC], F32R, kind="ExternalInput")
    w1_d = nc.dram_tensor("w1", [D, HID], F32R, kind="ExternalInput")
    w2_d = nc.dram_tensor("w2", [HID, D], F32R, kind="ExternalInput")
    wu_d = nc.dram_tensor("wu", [ER, D], F32R, kind="ExternalInput")
    b1_d = nc.dram_tensor("b1r", [128, HC], F32, kind="ExternalInput")
    b2_d = nc.dram_tensor("b2r", [128, MC], F32, kind="ExternalInput")
    rwd_d = nc.dram_tensor("rwd", [D, 56], F32, kind="ExternalInput")
    rb_d = nc.dram_tensor("rb", [E, 1], F32, kind="ExternalInput")
    bx_d = nc.dram_tensor("bexp", [E, ER], F32R, kind="ExternalInput")
    id_d = nc.dram_tensor("ident", [128, 128], F32, kind="ExternalInput")
    out_d = nc.dram_tensor("outT", [D, TC], F32, kind="ExternalOutput")

    with tile.TileContext(nc) as tc:
        with (
            tc.tile_pool(name="const", bufs=1) as cp,
            tc.tile_pool(name="big", bufs=1) as bp,
            tc.tile_pool(name="wts", bufs=2) as wp,
            tc.tile_pool(name="hbuf", bufs=3) as hp,
            tc.tile_pool(name="small", bufs=2) as sp,
        ):
            # ---- resident loads ----
            # small constants go on the gpsimd DMA queue so they don't sit
            # behind the 4.8MB xt stream on the sync queue
            rwd = cp.tile([128, DC * 56], F32, tag="rwd")
            nc.gpsimd.dma_start(
                rwd[:].rearrange("p (c e) -> p c e", c=DC),
                rwd_d.rearrange("(c p) e -> p c e", p=128),
            )
            wu = cp.tile([ER, D], F32R, tag="wu")
            nc.gpsimd.dma_start(wu[:], wu_d[:])
            b1 = cp.tile([128, HC], F32, tag="b1")
            nc.gpsimd.dma_start(b1[:], b1_d[:])
            b2 = cp.tile([128, MC], F32, tag="b2")
            nc.gpsimd.dma_start(b2[:], b2_d[:])
            rb = cp.tile([E, 1], F32, tag="rb")
            nc.gpsimd.dma_start(rb[:], rb_d[:])
            bx = cp.tile([E, ER], F32R, tag="bx")
            nc.gpsimd.dma_start(bx[:], bx_d[:])
            ident = cp.tile([128, 128], F32, tag="ident")
            nc.gpsimd.dma_start(ident[:], id_d[:])

            # xt as one tile per token tile so deps are per token range
            xts = []
            t0 = 0
            for i, n in enumerate(NT_SIZES):
                x_i = bp.tile([128, DC * n], F32R, tag=f"xt{i}",
                              name=f"xt{i}")
                h = n // 2
                for lo, hi in ((0, h), (h, n)):
                    nc.sync.dma_start(
                        x_i[:].rearrange("p (c t) -> p c t", c=DC)[:, :, lo:hi],
                        xt_d.rearrange("(c p) t -> p c t", p=128)
                            [:, :, t0 + lo:t0 + hi],
                    )
                xts.append(x_i)
                t0 += n

            lgT = bp.tile([E, TC], F32, tag="lgT")
            comb_t = bp.tile([E, TC], F32R, tag="combt")
            scaled = bp.tile([ER, TC], F32R, tag="scaled")
            acc = bp.tile([128, MC * TC], F32, tag="acc")

            # ---- phase A: router (exact f32) + LoRA expert activations ----
            # PE order: logits MMs -> logit transposes -> LoRA down MMs ->
            # comb transposes -> expand MMs. The DVE softmax chains overlap
            # the LoRA matmuls so the PE never waits on them.
            lgtok = bp.tile([128, 3 * len(RC_SIZES)], F32, tag="lgtok")
            combtok = bp.tile([128, 3 * len(RC_SIZES)], F32, tag="combtok")
            acts = bp.tile([ER, TC], F32, tag="acts")
            with tc.tile_pool(name="psA", bufs=1, space="PSUM") as psA:
                # merged router logits + LoRA down, exact f32:
                # [3+24, n] = [rw | wd]^T @ xT
                t0 = 0
                for i, n in enumerate(NT_SIZES):
                    dn27 = psA.tile([56, 512], F32, tag="dn27", bufs=2,
                                    name=f"dn27_{t0}")
                    for c in range(DC):
                        nc.tensor.matmul(
                            dn27[:, :n],
                            rwd[:, c * 56:(c + 1) * 56],
                            xts[i][:, c * n:(c + 1) * n].bitcast(F32),
                            start=(c == 0), stop=(c == DC - 1),
                        )
                    nc.vector.tensor_scalar_add(lgT[:, t0:t0 + n],
                                                dn27[:E, :n], rb[:])
                    nc.scalar.activation(acts[:, t0:t0 + n], dn27[32:, :n],
                                         AF.Gelu)
                    t0 += n

                # transpose logits to token-major, 128-token chunks
                t0 = 0
                for ci, n in enumerate(RC_SIZES):
                    lg = psA.tile([128, E], F32, tag="lg", bufs=2,
                                  name=f"lg_{t0}")
                    nc.tensor.transpose(lg[:n, :], lgT[:, t0:t0 + n],
                                        ident[:E, :E])
                    nc.vector.tensor_copy(lgtok[:n, 3 * ci:3 * ci + 3],
                                          lg[:n, :])
                    t0 += n

            # ---- phase B: base MLP + LoRA-up, hidden dim in 4 groups ----
            with (
                tc.tile_pool(name="psO", bufs=1, space="PSUM") as psO,
                tc.tile_pool(name="psH", bufs=2, space="PSUM") as psH,
            ):
                for g in range(NGROUPS):
                    w1g = wp.tile([128, DC * GH * 128], F32R, tag="w1g")
                    gw = GH * 128
                    for lo, hi in ((0, gw // 2), (gw // 2, gw)):
                        nc.sync.dma_start(
                            w1g[:].rearrange("p (c f) -> p c f", c=DC)
                               [:, :, lo:hi],
                            w1_d.rearrange("(c p) f -> p c f", p=128)
                                [:, :, g * gw + lo:g * gw + hi],
                        )
                    w2g = wp.tile([128, GH * D], F32R, tag="w2g")
                    for lo, hi in ((0, GH // 2), (GH // 2, GH)):
                        nc.sync.dma_start(
                            w2g[:].rearrange("p (c f) -> p c f", c=GH)
                               [:, lo:hi, :],
                            w2_d.rearrange("(c p) f -> p c f", p=128)
                                [:, g * GH + lo:g * GH + hi, :],
                        )
                    t0 = 0
                    for nt, n in enumerate(NT_SIZES):
                        outp = [psO.tile([128, 512], F32, tag=f"out{m}",
                                         name=f"out{m}_{g}_{nt}")
                                for m in range(MC)]
                        hsb = [None] * GH
                        for j in range(GH + 1):
                            if j < GH:
                                hps = psH.tile([128, 512], F32, tag="h",
                                               name=f"h_{g}_{nt}_{j}")
                                for c in range(DC):
                                    nc.tensor.matmul(
                                        hps[:, :n],
                                        w1g[:, (c * GH + j) * 128:
                                               (c * GH + j) * 128 + 128],
                                        xts[nt][:, c * n:(c + 1) * n],
                                        start=(c == 0), stop=(c == DC - 1),
                                    )
                                hsb[j] = hp.tile([128, 512], F32R, tag="hs",
                                                 name=f"hs_{g}_{nt}_{j}")
                                nc.scalar.activation(
                                    hsb[j][:, :n], hps[:, :n], AF.Gelu,
                                    bias=b1[:, g * GH + j:g * GH + j + 1],
                                )
                            if j >= 1:
                                jj = j - 1
                                for m in range(MC):
                                    nc.tensor.matmul(
                                        outp[m][:, :n],
                                        w2g[:, jj * D + m * 128:
                                               jj * D + m * 128 + 128],
                                        hsb[jj][:, :n],
                                        start=(jj == 0),
                                        stop=(jj == GH - 1 and g < NGROUPS - 1),
                                    )
                        if g == NGROUPS - 1:
                            for m in range(MC):
                                nc.tensor.matmul(
                                    outp[m][:, :n],
                                    wu[:, m * 128:(m + 1) * 128],
                                    scaled[:, t0:t0 + n],
                                    start=False, stop=True,
                                )
                        for m in range(MC):
                            a = acc[:, m * TC + t0:m * TC + t0 + n]
                            if g == 0:
                                nc.vector.tensor_copy(a, outp[m][:, :n])
                            elif g < NGROUPS - 1:
                                nc.vector.tensor_add(a, a, outp[m][:, :n])
                            else:
                                nc.vector.scalar_tensor_tensor(
                                    a, outp[m][:, :n], b2[:, m:m + 1], a,
                                    op0=ALU.add, op1=ALU.add,
                                )
                                nc.sync.dma_start(
                                    out_d[m * 128:(m + 1) * 128, t0:t0 + n], a)
                        t0 += n

                    if g == 0:
                        # softmax + top-2 combine weights; DVE is free now,
                        # all probs from one table-stable Exp (logits are
                        # O(1), no max-subtraction needed in fp32)
                        probs = bp.tile([128, 3 * len(RC_SIZES)], F32,
                                        tag="probs")
                        nc.scalar.activation(probs[:], lgtok[:], AF.Exp)
                        tq = 0
                        for ci, n in enumerate(RC_SIZES):
                            pr = probs[:, 3 * ci:3 * ci + 3]
                            ssum = sp.tile([128, 1], F32, tag="ssum",
                                           name=f"ssum_{tq}")
                            nc.vector.tensor_reduce(ssum[:n], pr[:n, :],
                                                    axis=AX.X, op=ALU.add)
                            pmin = sp.tile([128, 1], F32, tag="pmin",
                                           name=f"pmin_{tq}")
                            nc.vector.tensor_reduce(pmin[:n], pr[:n, :],
                                                    axis=AX.X, op=ALU.min)
                            rs = sp.tile([128, 1], F32, tag="rs",
                                         name=f"rs_{tq}")
                            nc.vector.reciprocal(rs[:n], ssum[:n])
                            d0 = sp.tile([128, 1], F32, tag="d0",
                                         name=f"d0_{tq}")
                            nc.vector.tensor_sub(d0[:n], ssum[:n], pmin[:n])
                            den = sp.tile([128, 1], F32, tag="den",
                                          name=f"den_{tq}")
                            nc.vector.tensor_scalar(den[:n], d0[:n], rs[:n],
                                                    1e-6, op0=ALU.mult,
                                                    op1=ALU.add)
                            invd = sp.tile([128, 1], F32, tag="invd",
                                           name=f"invd_{tq}")
                            nc.vector.reciprocal(invd[:n], den[:n])
                            t1 = sp.tile([128, 1], F32, tag="t1",
                                         name=f"t1_{tq}")
                            nc.vector.tensor_mul(t1[:n], rs[:n], invd[:n])
                            mask = sp.tile([128, E], F32, tag="mask",
                                           name=f"mask_{tq}")
                            nc.vector.tensor_scalar(mask[:n, :], pr[:n, :],
                                                    pmin[:n], None,
                                                    op0=ALU.is_gt)
                            nc.vector.scalar_tensor_tensor(
                                combtok[:n, 3 * ci:3 * ci + 3], mask[:n, :],
                                t1[:n], pr[:n, :], op0=ALU.mult, op1=ALU.mult,
                            )
                            tq += n

                    if g == 1:
                        # comb transposes + expand, overlapped behind group 1
                        tq = 0
                        for ci, n in enumerate(RC_SIZES):
                            tp = psH.tile([E, 128], F32, tag="h",
                                          name=f"tp_{tq}")
                            nc.tensor.transpose(tp[:, :n],
                                                combtok[:n, 3 * ci:3 * ci + 3],
                                                ident[:n, :n])
                            nc.vector.tensor_copy(comb_t[:, tq:tq + n],
                                                  tp[:, :n])
                            tq += n
                        tq = 0
                        for i, n in enumerate(NT_SIZES):
                            ex = psH.tile([ER, 512], F32, tag="h",
                                          name=f"ex_{tq}")
                            nc.tensor.matmul(ex[:, :n], bx[:],
                                             comb_t[:, tq:tq + n],
                                             start=True, stop=True)
                            nc.vector.tensor_mul(scaled[:, tq:tq + n],
                                                 acts[:, tq:tq + n], ex[:, :n])
                            tq += n

    nc.compile()
    return nc


def _pack_rwd(router_w, w_down):
    rwd = np.zeros((D, 56), np.float32)
    rwd[:, :E] = np.asarray(router_w, np.float32)
    rwd[:, 32:] = np.asarray(w_down, np.float32).transpose(1, 0, 2).reshape(D, ER)
    return rwd


def _prep_inputs(x, w1, b1, w2, b2, router_w, router_b, w_down, w_up):
    x = np.ascontiguousarray(np.asarray(x, dtype=np.float32))
    xT = x.reshape(T, D).T  # [D, T]
    common = {
        "w1": np.ascontiguousarray(np.asarray(w1, np.float32)),
        "w2": np.ascontiguousarray(np.asarray(w2, np.float32)),
        "wu": np.ascontiguousarray(np.asarray(w_up, np.float32).reshape(ER, D)),
        "b1r": np.ascontiguousarray(
            np.asarray(b1, np.float32).reshape(HC, 128).T),
        "b2r": np.ascontiguousarray(
            np.asarray(b2, np.float32).reshape(MC, 128).T),
        "rwd": _pack_rwd(router_w, w_down),
        "rb": np.ascontiguousarray(
            np.asarray(router_b, np.float32).reshape(E, 1)),
        "bexp": np.repeat(np.eye(E, dtype=np.float32), R, axis=1),
        "ident": np.eye(128, dtype=np.float32),
    }
    in_maps = []
    for c in range(NCORES):
        m = dict(common)
        m["xt"] = np.ascontiguousarray(xT[:, c * TC:(c + 1) * TC])
        in_maps.append(m)
    return in_maps


def _run(inputs, trace=False):
    if "nc" not in _cache:
        _cache["nc"] = _build()
    nc = _cache["nc"]
    in_maps = _prep_inputs(**inputs)
    res = run_bass_kernel_spmd(nc, in_maps, core_ids=list(range(NCORES)),
                               trace=trace)
    outT = np.concatenate([res.results[c]["outT"] for c in range(NCORES)],
                          axis=1)  # [D, T]
    out = np.ascontiguousarray(outT.T).reshape(B, N, D).astype(np.float32)
    return out, res


def kernel(**inputs):
    return _run(inputs)[0]
